# revision 24
# baseline (speedup 1.0000x reference)
"""Trainium2 Bass kernel for BackprojectDepth.

out[b, i, y*W+x] = depth[b, 0, y, x] * (A[b,i]*(x+dx[b]) + B[b,i]*(y+dy[b]) + C[b,i])  for i in 0..2
out[b, 3, :]    = 1.0

Sharding: pure data parallel over batch (32 batches -> 4 per core on 8 cores).

Device I/O is fp16: the 2e-2 relative-error budget dwarfs fp16's ~1e-3
roundoff, |out| <= ~3e3 is far inside fp16 range, and x-coords < 2048
are exactly representable.  (8-bit I/O was tried and abandoned: any
int8/uint8 operand or destination drops DVE ops out of the 2x packed
fp16 mode and costs more than the DMA bytes it saves.)  The constant
ones plane (out[:,3,:]) is filled host-side like the other
host-precomputed constants, so per-core traffic is 4 MB depth in +
12 MB cam planes out.

Measured TRN2 facts this schedule is built on (all from ntff traces of
earlier revisions of this kernel):
 * a dma_start occupies the issuing engine queue ~0.6 us regardless of
   size up to ~1 MB -> use whole-plane [128, 4096] (1 MB) transfers;
 * only sync/scalar/gpsimd can issue DMAs; an out-DMA placed on the
   scalar queue between ACT ops stalls ACT until DVE finishes that
   plane, so outputs go on the otherwise-idle sync queue only;
 * DVE: tensor_scalar 539 ns, tensor_tensor 685 ns per [128,1024] fp16
   (2x packed mode), scalar_tensor_tensor always 1x (1279 ns) - so the
   affine term is built by TS/ACT and multiplied by a wide TT;
 * ACT: any affine op is 1233 ns per [128,1024] regardless of dtype;
 * every engine pays a ~7 us framework preamble before its first real
   instruction, so the first compute can't start before ~8 us.

Per (b, plane): a [128, 4096] lin tile is filled by 4 affine sub-ops
(lin[p, t*1024+m] = A*m + B*(128t+p) + A*dx+B*dy+C, per-partition
scale/bias vectors), split between DVE tensor_scalar ('V') and ACT
activation ('A') by sub_pattern; one wide tensor_tensor multiplies by
the per-batch depth tile (2290 ns vs 4x685 narrow); the plane leaves as
one 1 MB DMA on sync.  mul_pattern can move whole plane-multiplies to
the gpsimd/Pool engine ('P').  Depth loads ride gpsimd; the f32
scale/bias tables ride scalar ahead of its ACT stream; a dummy
activation right after engine init pulls the ACT_TABLE_LOAD off the
critical path.
"""

import numpy as np

import concourse.tile as tile
from concourse import bacc, mybir
from concourse.bass_utils import run_bass_kernel_spmd

N_CORES = 8
B, H, W = 32, 512, 1024
HW = H * W
BPC = B // N_CORES          # batches per core
TPB = H // 128              # row-tiles per batch (partition dim = 128 rows)
WB = W * TPB                # free dim of a whole-plane [128, 4096] tile

F32 = mybir.dt.float32
F16 = mybir.dt.float16

_TRACE = False              # test.py may flip this for profiling
_LAST_RESULTS = None        # BassKernelResults from the last run (for test.py)

_nc_cache = None

# tuning knobs (resolved defaults)
DEFAULT_CFG = dict(
    # sub_pattern[b*3+i][t]: engine for each affine sub-op: V=DVE TS, A=ACT,
    # G=gpsimd/Pool TS.  'G' works (vector-scalar mult+add compiles and is
    # correct on Pool) but measured 2390 ns/subtile AND its streaming
    # contends with DVE's packed reads, inflating DVE tensor_scalar from 539
    # to 1915 ns -- so Pool gets no affine work.  Whole planes stay
    # single-producer so each wide multiply joins only one upstream engine
    # chain; the first plane of each batch is DVE-made so the vector engine
    # is never gated on ACT at batch starts.
    sub_pattern=(
        "VVVV", "AAAA", "AAAA",
        "VVVV", "AAAA", "AAAA",
        "VVVV", "AAAA", "AAAA",
        "VVVV", "VVVV", "AAAA",
    ),
    # mul_pattern[b*3+i]: engine for that plane's wide multiply: V=DVE, P=Pool
    # (Pool wide TT measured 8078 ns vs 2290 DVE and contends for SBUF: unused)
    mul_pattern="VVVVVVVVVVVV",
    warmup=True,            # dummy activation to preload ACT tables early
    depth_ring="gpsimd",
    out_ring="sync",
    dpool=3, lpoolv=3, lpoola=4, lpoolg=3, opool=4,
)


def _build(**cfg_over):
    """Build + compile the per-core Bass program (SPMD: same NEFF, 8 cores)."""
    cfg = dict(DEFAULT_CFG, **cfg_over)
    nc = bacc.Bacc(
        "TRN2",
        target_bir_lowering=False,
        debug=False,
        enable_asserts=False,
        num_devices=N_CORES,
    )

    depth_d = nc.dram_tensor("depth", [BPC, H, W], F16, kind="ExternalInput")
    xg_d = nc.dram_tensor("xg", [128, W], F16, kind="ExternalInput")
    # scalar operands of tensor_scalar/activation must stay f32
    scale_d = nc.dram_tensor("scale", [128, BPC * 3], F32, kind="ExternalInput")
    bias_d = nc.dram_tensor("bias", [128, BPC * 3 * TPB], F32, kind="ExternalInput")
    out_d = nc.dram_tensor("out", [BPC, 3, HW], F16, kind="ExternalOutput")

    engines = {"sync": nc.sync, "scalar": nc.scalar, "gpsimd": nc.gpsimd}

    with tile.TileContext(nc) as tc:
        with (
            tc.tile_pool(name="const", bufs=1) as cpool,
            tc.tile_pool(name="dpool", bufs=cfg["dpool"]) as dpool,
            tc.tile_pool(name="lpoolv", bufs=cfg["lpoolv"]) as lpoolv,
            tc.tile_pool(name="lpoola", bufs=cfg["lpoola"]) as lpoola,
            tc.tile_pool(
                name="lpoolg",
                bufs=cfg["lpoolg"] if any("G" in p for p in cfg["sub_pattern"]) else 1,
            ) as lpoolg,
            tc.tile_pool(name="opool", bufs=cfg["opool"]) as opool,
        ):
            # scale rides scalar; x-ramp + bias ride sync.  (Putting all
            # three consts plus the first depth tile on the scalar ring was
            # tried to dodge the ~6.7 us first-transfer completion latency,
            # and was SLOWER -- the latency is per-ring pipeline depth, and
            # serializing the consts behind each other delays the last one;
            # spreading them across two rings is the faster variant.)
            sc_t = cpool.tile([128, BPC * 3], F32)
            nc.scalar.dma_start(sc_t[:], scale_d.ap())
            xg_t = cpool.tile([128, W], F16)
            nc.sync.dma_start(xg_t[:], xg_d.ap())
            bi_t = cpool.tile([128, BPC * 3 * TPB], F32)
            nc.sync.dma_start(bi_t[:], bias_d.ap())
            if cfg["warmup"]:
                wu_t = cpool.tile([128, BPC * 3], F32)
                nc.scalar.activation(
                    wu_t[:], sc_t[:], mybir.ActivationFunctionType.Identity
                )

            # out[b, i, t*131072 + p*1024 + m] ; depth[b, (t*128+p)*1024 + m]
            out_ap = out_d.ap().rearrange("b i (t p m) -> b i p t m", t=TPB, p=128)
            depth_ap = depth_d.ap().rearrange("b (t p) m -> b p t m", p=128)

            # NOTE: a one-batch software-pipeline skew (all of batch b's lin
            # ops issued before batch b-1's multiplies) was tried and was
            # SLOWER: with every engine saturated simultaneously, per-op
            # throughput dropped ~15-20% (ACT 1188->1431 ns, wide TT
            # 2284->2738 ns) from memory-port contention.  The plane-serial
            # order below keeps concurrency moderate and is faster end-to-end.
            for b in range(BPC):
                d_t = dpool.tile([128, WB], F16)
                engines[cfg["depth_ring"]].dma_start(
                    d_t[:].rearrange("p (t m) -> p t m", t=TPB), depth_ap[b]
                )
                for i in range(3):
                    col = 3 * b + i
                    # separate pools per producer so a stalled consumer of an
                    # ACT-made tile never blocks DVE tile allocation (and v.v.)
                    pat = cfg["sub_pattern"][col]
                    lin = {
                        "V": lpoolv, "A": lpoola, "G": lpoolg
                    }[pat[0]].tile([128, WB], F16)
                    for t in range(TPB):
                        seg = lin[:, t * W : (t + 1) * W]
                        bias_col = bi_t[:, col * TPB + t : col * TPB + t + 1]
                        if pat[t] == "A":
                            nc.scalar.activation(
                                seg,
                                xg_t[:],
                                mybir.ActivationFunctionType.Identity,
                                bias=bias_col,
                                scale=sc_t[:, col : col + 1],
                            )
                        else:
                            eng = nc.vector if pat[t] == "V" else nc.gpsimd
                            eng.tensor_scalar(
                                seg,
                                xg_t[:],
                                sc_t[:, col : col + 1],
                                bias_col,
                                mybir.AluOpType.mult,
                                mybir.AluOpType.add,
                            )
                    o_t = opool.tile([128, WB], F16)
                    meng = nc.vector if cfg["mul_pattern"][col] == "V" else nc.gpsimd
                    meng.tensor_mul(o_t[:], lin[:], d_t[:])
                    engines[cfg["out_ring"]].dma_start(
                        out_ap[b, i],
                        o_t[:].rearrange("p (t m) -> p t m", t=TPB),
                    )

    nc.compile()
    return nc


def _make_in_maps(depth, inv_K, dxy):
    depth16 = np.asarray(depth, dtype=np.float32)[:, 0].astype(np.float16)  # [B,H,W]
    K = np.asarray(inv_K, dtype=np.float64)
    dx = np.asarray(dxy, dtype=np.float64)

    # Per-batch affine coefficients: cam_i = A*x' + B*y' + C with x'=x+dx, y'=y+dy
    A = K[:, :3, 0]                                   # [B, 3]
    Bc = K[:, :3, 1]
    C = K[:, :3, 2]
    const = A * dx[:, None, 0] + Bc * dx[:, None, 1] + C   # [B, 3]

    p = np.arange(128, dtype=np.float64)
    yrow = 128.0 * np.arange(TPB, dtype=np.float64)[:, None] + p[None, :]  # [TPB,128]
    # bias[g, i, t, p] = B*(128t+p) + const
    bias_all = Bc[:, :, None, None] * yrow[None, None] + const[:, :, None, None]

    xg = np.ascontiguousarray(
        np.broadcast_to(np.arange(W, dtype=np.float16), (128, W))
    )

    in_maps = []
    for c in range(N_CORES):
        g0 = c * BPC
        bias_c = np.ascontiguousarray(
            bias_all[g0 : g0 + BPC]                  # [BPC, 3, TPB, 128]
            .reshape(BPC * 3 * TPB, 128)
            .T.astype(np.float32)
        )                                            # [128, BPC*3*TPB]
        scale_c = np.ascontiguousarray(
            np.broadcast_to(
                A[g0 : g0 + BPC].reshape(BPC * 3).astype(np.float32),
                (128, BPC * 3),
            )
        )
        in_maps.append(
            {
                "depth": np.ascontiguousarray(depth16[g0 : g0 + BPC]),
                "scale": scale_c,
                "bias": bias_c,
                "xg": xg,
            }
        )
    return in_maps


def _expected_inputs(nc):
    import concourse.mybir as _mybir

    names = set()
    for alloc in nc.m.functions[0].allocations:
        if (
            isinstance(alloc, _mybir.MemoryLocationSet)
            and alloc.kind == "ExternalInput"
        ):
            names.add(alloc.memorylocations[0].name)
    return names


def _run(nc, in_maps, trace=False):
    global _LAST_RESULTS
    want = _expected_inputs(nc)
    in_maps = [{k: v for k, v in m.items() if k in want} for m in in_maps]
    res = run_bass_kernel_spmd(
        nc, in_maps, core_ids=list(range(N_CORES)), trace=trace
    )
    _LAST_RESULTS = res
    out = np.empty((B, 4, HW), dtype=np.float32)
    for c in range(N_CORES):
        out[c * BPC : (c + 1) * BPC, :3] = res.results[c]["out"].astype(np.float32)
    out[:, 3] = 1.0
    return out


def kernel(depth, inv_K, dxy):
    global _nc_cache
    in_maps = _make_in_maps(depth, inv_K, dxy)
    if _nc_cache is None:
        _nc_cache = _build()
    return _run(_nc_cache, in_maps, trace=_TRACE)
